# revision 47
# baseline (speedup 1.0000x reference)
"""GATv2 (3-layer, PyG semantics) + global mean pool + MLP on 8 trn2 NeuronCores.

Self-contained: hardcodes problem shapes from nn_GAT_47906065219807.
Sharding: data-parallel over contiguous node ranges (2500 nodes/core); each
core owns edges whose dst lands in its range (edges cross shards via an
AllGather of the source-side projections).

v2 edge phase (per 128-dst-node window, chunks of <=4 edge tiles):
  - ONE merged dma_gather per chunk pulls both xl[src] and xr[dst] rows from a
    unified DRAM buffer (AllGathered xl ++ locally-written xr).
  - s = gx + gr (one batched TT), p = prelu(s) (one batched ACT op)
  - score[e,h] = sum_c att[h,c] * p[e,h,c] via scalar_tensor_tensor accum_out
    (fuses the att multiply and the segmented reduce)
  - exp with a stride-0-broadcast input writes ex replicated 128-wide, so the
    per-head mask scaling is ONE 2x-mode TT per chunk
  - den via per-tile matmul (binary mask x ex), aggregation via per-(tile,head)
    matmul (scaled mask x raw gx) accumulated in PSUM
"""
import numpy as np
import ml_dtypes

import concourse.bacc as bacc
import concourse.mybir as mybir
import concourse.tile as tile
from concourse.bass_utils import run_bass_kernel_spmd

# problem constants
N_NODES = 20000
N_EDGES = 120000
N_GRAPHS = 512
F_IN = 300
NHID = 256
NOUT = 768
SLOPE = 0.2
EPS = 1e-16

NCORES = 8
NLOC = N_NODES // NCORES            # 2500
NPAD = 2560                         # 20 tiles of 128
NWIN = NPAD // 128                  # 20 windows / node tiles per core
KIN_PAD = 384                       # F_IN padded to 3*128
GR_BASE = NCORES * NPAD             # total rows in xl_full
CT = 4                              # edge tiles per gather chunk
NQ = 4                              # projection/AllGather chunks per layer
QROWS = NPAD // NQ                  # 640 rows per chunk
QTILES = QROWS // 128               # 5 node tiles per chunk

# per-layer dims: (K_in_padded, F_out, heads, concat)
LAYERS = [
    (KIN_PAD, 4 * NHID, 4, True),
    (4 * NHID, 4 * NHID, 4, True),
    (4 * NHID, 6 * NHID, 6, False),
]

_BF = ml_dtypes.bfloat16
_PROG_CACHE = {}
DEBUG = False


def _bf16(a):
    return np.ascontiguousarray(a.astype(_BF)).view(np.uint16)


def _wrap_idx(flat_idx):
    """dma_gather index layout: slot i -> [i % 16, i // 16], replicated to
    128 partitions (8 Q7 cores x 16)."""
    n = flat_idx.shape[0]
    assert n % 16 == 0
    w = flat_idx.reshape(n // 16, 16).T.astype(np.int16)
    return np.tile(w, (8, 1)).copy()


def _preprocess(inputs):
    """Host-side sharding/layout. Returns (Tw, in_maps)."""
    x = np.asarray(inputs["x"], np.float32)
    ei = np.asarray(inputs["edge_index"]).astype(np.int64)
    batch = np.asarray(inputs["batch"]).astype(np.int64)

    loops = np.arange(N_NODES, dtype=np.int64)
    src = np.concatenate([ei[0], loops])
    dst = np.concatenate([ei[1], loops])

    # row index into xl_full, whose layout is chunk-major:
    # [quarter q][core c][row r within quarter]  (AllGather chunk per quarter)
    s_core = src // NLOC
    s_loc = src % NLOC
    s_q = s_loc // QROWS
    src_pad = s_q * (NCORES * QROWS) + s_core * QROWS + (s_loc - s_q * QROWS)

    core_of = dst // NLOC
    buckets = [[[] for _ in range(NWIN)] for _ in range(NCORES)]
    order = np.argsort(dst, kind="stable")
    for e in order:
        c = core_of[e]
        dl = dst[e] - c * NLOC
        buckets[c][dl // 128].append(e)
    Tw = tuple(
        max((len(buckets[c][w]) + 127) // 128 for c in range(NCORES))
        for w in range(NWIN)
    )
    Soff = np.concatenate([[0], np.cumsum([t * 128 for t in Tw])])
    TOT = int(Soff[-1])

    # graph counts -> reciprocal (reference divides by max(cnt,1))
    cnt = np.bincount(batch, minlength=N_GRAPHS).astype(np.float32)
    rcnt = 1.0 / np.maximum(cnt, 1.0)

    def wT_pad(w, kpad):
        # host layout [128, KB, F]: [p, b, f]; w is [F, K]
        wt = w.T.astype(np.float32)                    # [K, F]
        K, F = wt.shape
        out = np.zeros((kpad, F), np.float32)
        out[:K] = wt
        return _bf16(out.reshape(kpad // 128, 128, F).transpose(1, 0, 2)
                     .reshape(128, (kpad // 128) * F))

    shared = {
        "w1l": wT_pad(np.asarray(inputs["c1_wl"]), KIN_PAD),
        "w1r": wT_pad(np.asarray(inputs["c1_wr"]), KIN_PAD),
        "w2l": wT_pad(np.asarray(inputs["c2_wl"]), 4 * NHID),
        "w2r": wT_pad(np.asarray(inputs["c2_wr"]), 4 * NHID),
        "w3l": wT_pad(np.asarray(inputs["c3_wl"]), 4 * NHID),
        "w3r": wT_pad(np.asarray(inputs["c3_wr"]), 4 * NHID),
        "att1": _bf16(np.tile(np.asarray(inputs["c1_att"]).reshape(1, -1), (128, 1))),
        "att2": _bf16(np.tile(np.asarray(inputs["c2_att"]).reshape(1, -1), (128, 1))),
        "att3": _bf16(np.tile(np.asarray(inputs["c3_att"]).reshape(1, -1), (128, 1))),
        "b1": _bf16(np.tile(np.asarray(inputs["c1_b"], np.float32).reshape(1, -1), (128, 1))),
        "b2": _bf16(np.tile(np.asarray(inputs["c2_b"], np.float32).reshape(1, -1), (128, 1))),
        "b3": np.tile(np.asarray(inputs["c3_b"], np.float32).reshape(1, -1), (128, 1)),
        "rcnt": np.tile(rcnt.reshape(1, -1), (128, 1)).astype(np.float32),
        "wfc1": wT_pad(np.asarray(inputs["fp1_w"]), 256),
        "wfc2": wT_pad(np.asarray(inputs["fp2_w"]), 256),
        "bfc1": np.asarray(inputs["fp1_b"], np.float32).reshape(2, 128).T.copy(),
        "bfc2": np.tile(np.asarray(inputs["fp2_b"], np.float32).reshape(1, -1),
                        (128, 1)),
    }

    in_maps = []
    for c in range(NCORES):
        xc = np.zeros((NPAD, KIN_PAD), np.float32)
        xc[:NLOC, :F_IN] = x[c * NLOC:(c + 1) * NLOC]
        xT = xc.T.reshape(KIN_PAD // 128, 128, NPAD).transpose(1, 0, 2)
        xT = _bf16(xT.reshape(128, (KIN_PAD // 128) * NPAD))

        # gather indices: window-contiguous src rows (into xl_full) and
        # dst rows (into xr_loc)
        isrc = np.zeros(TOT, np.int64)
        idst = np.zeros(TOT, np.int64)
        emask = np.zeros((128, TOT), np.float32)
        for w in range(NWIN):
            es = buckets[c][w]
            s0 = int(Soff[w])
            # emask: slot (tile t, partition p) with dst row n ->
            # emask[p, s0 + t*128 + n] = 1
            for i, e in enumerate(es):
                n = (dst[e] - c * NLOC) - w * 128
                emask[i % 128, s0 + (i // 128) * 128 + n] = 1.0
                isrc[s0 + i] = src_pad[e]
                idst[s0 + i] = dst[e] - c * NLOC

        pmask = np.zeros((128, NWIN * N_GRAPHS), np.float32)
        bl = batch[c * NLOC:(c + 1) * NLOC]
        for nl in range(NLOC):
            pmask[nl % 128, (nl // 128) * N_GRAPHS + bl[nl]] = 1.0

        m = dict(shared)
        m["xT"] = xT
        m["isrc"] = _wrap_idx(isrc)
        m["idst"] = _wrap_idx(idst)
        m["emask"] = _bf16(emask)
        m["pmask"] = _bf16(pmask)
        in_maps.append(m)
    return Tw, in_maps


def _build(Tw):
    Tw = tuple(Tw)
    TMAX = max(Tw)
    Soff = [0]
    for t in Tw:
        Soff.append(Soff[-1] + t * 128)
    TOT = Soff[-1]
    nc = bacc.Bacc("TRN2", target_bir_lowering=False, debug=False,
                   num_devices=NCORES)
    dt = mybir.dt
    AF = mybir.ActivationFunctionType
    OP = mybir.AluOpType

    def inp(name, shape, d):
        return nc.dram_tensor(name, shape, d, kind="ExternalInput")

    xT_in = inp("xT", [128, (KIN_PAD // 128) * NPAD], dt.bfloat16)
    isrc_in = inp("isrc", [128, TOT // 16], dt.int16)
    idst_in = inp("idst", [128, TOT // 16], dt.int16)
    emask_in = inp("emask", [128, TOT], dt.bfloat16)
    pmask_in = inp("pmask", [128, NWIN * N_GRAPHS], dt.bfloat16)
    w_in = [(inp("w1l", [128, 3 * 1024], dt.bfloat16),
             inp("w1r", [128, 3 * 1024], dt.bfloat16)),
            (inp("w2l", [128, 8 * 1024], dt.bfloat16),
             inp("w2r", [128, 8 * 1024], dt.bfloat16)),
            (inp("w3l", [128, 8 * 1536], dt.bfloat16),
             inp("w3r", [128, 8 * 1536], dt.bfloat16))]
    att_in = [inp("att1", [128, 1024], dt.bfloat16),
              inp("att2", [128, 1024], dt.bfloat16),
              inp("att3", [128, 1536], dt.bfloat16)]
    b_in = [inp("b1", [128, 1024], dt.bfloat16),
            inp("b2", [128, 1024], dt.bfloat16),
            inp("b3", [128, 256], dt.float32)]
    rcnt_in = inp("rcnt", [128, N_GRAPHS], dt.float32)
    wfc1_in = inp("wfc1", [128, 2 * 256], dt.bfloat16)
    wfc2_in = inp("wfc2", [128, 2 * 768], dt.bfloat16)
    bfc1_in = inp("bfc1", [128, 2], dt.float32)
    bfc2_in = inp("bfc2", [128, 768], dt.float32)
    out_ext = nc.dram_tensor("out", [N_GRAPHS, NOUT], dt.float32,
                             kind="ExternalOutput")
    if DEBUG:
        dbg_xg0 = nc.dram_tensor("dbg_xg0", [GR_BASE + NPAD, 1024],
                                 dt.bfloat16, kind="ExternalOutput")
        dbg_h0 = nc.dram_tensor("dbg_h0", [NPAD, 1024], dt.bfloat16,
                                kind="ExternalOutput")
        dbg_h1 = nc.dram_tensor("dbg_h1", [NPAD, 1024], dt.bfloat16,
                                kind="ExternalOutput")
        dbg_pool = nc.dram_tensor("dbg_pool", [256, N_GRAPHS], dt.float32,
                                  kind="ExternalOutput")
        dbg_den = nc.dram_tensor("dbg_den", [128, 6], dt.float32,
                                 kind="ExternalOutput")
        dbg_z = nc.dram_tensor("dbg_z", [128, 1024], dt.float32,
                               kind="ExternalOutput")

    # internal DRAM
    xl_loc = [nc.dram_tensor(f"xl_loc{l}", [NPAD, F], dt.bfloat16)
              for l, (_, F, _, _) in enumerate(LAYERS)]
    xr_loc = [nc.dram_tensor(f"xr_loc{l}", [NPAD, F], dt.bfloat16)
              for l, (_, F, _, _) in enumerate(LAYERS)]
    xl_full = [nc.dram_tensor(f"xl_full{l}", [GR_BASE, F], dt.bfloat16,
                              addr_space="Shared")
               for l, (_, F, _, _) in enumerate(LAYERS)]
    h_dram = [nc.dram_tensor(f"h_dram{l}", [NPAD, 1024], dt.bfloat16)
              for l in range(2)]
    pool_loc = nc.dram_tensor("pool_loc", [256, N_GRAPHS], dt.float32)
    pool_full = nc.dram_tensor("pool_full", [256, N_GRAPHS], dt.float32,
                               addr_space="Shared")

    rg = [list(range(NCORES))]

    with tile.TileContext(nc) as tc:
        with (
            tc.tile_pool(name="persist", bufs=1) as ppool,
        ):
            isrc_t = ppool.tile([128, TOT // 16], dt.int16)
            nc.sync.dma_start(out=isrc_t[:, :], in_=isrc_in[:, :])
            idst_t = ppool.tile([128, TOT // 16], dt.int16)
            nc.sync.dma_start(out=idst_t[:, :], in_=idst_in[:, :])

            def proj_quarter(l, q, hT_q, hoff, wl_t, wr_t, psA, mmpool):
                """Project rows [q*QROWS, (q+1)*QROWS) of layer l's input;
                issue the AllGather chunk for the xl side."""
                K, F, _, _ = LAYERS[l]
                KB = K // 128
                NCH = F // 512
                for side, (wt, dst_dram) in enumerate(
                        ((wl_t, xl_loc[l]), (wr_t, xr_loc[l]))):
                    for tq in range(QTILES):
                        t = q * QTILES + tq
                        for ch in range(NCH):
                            ps = psA.tile([128, 512], dt.float32, tag="mmps")
                            for b in range(KB):
                                nc.tensor.matmul(
                                    ps[:, :],
                                    hT_q[:, b, hoff + tq * 128:
                                         hoff + (tq + 1) * 128],
                                    wt[:, b, ch * 512:(ch + 1) * 512],
                                    start=(b == 0), stop=(b == KB - 1))
                            ob = mmpool.tile([128, 512], dt.bfloat16,
                                             tag="mmout")
                            nc.scalar.copy(ob[:, :], ps[:, :])
                            nc.sync.dma_start(
                                out=dst_dram[t * 128:(t + 1) * 128,
                                             ch * 512:(ch + 1) * 512],
                                in_=ob[:, :])
                    if side == 0:
                        nc.gpsimd.collective_compute(
                            "AllGather", mybir.AluOpType.bypass,
                            replica_groups=rg,
                            ins=[xl_loc[l][q * QROWS:
                                           (q + 1) * QROWS, :].opt()],
                            outs=[xl_full[l][q * NCORES * QROWS:
                                             (q + 1) * NCORES *
                                             QROWS, :].opt()])

            # ---- initial projection: layer 0 (quartered, AG overlapped) ----
            with (
                tc.tile_pool(name="w0", bufs=1) as w0pool,
                tc.tile_pool(name="hT0", bufs=1) as h0pool,
                tc.tile_pool(name="mm0", bufs=4) as mm0pool,
                tc.tile_pool(name="psA0", bufs=2, space="PSUM") as psA0,
            ):
                KB0 = KIN_PAD // 128
                hT = h0pool.tile([128, KB0, NPAD], dt.bfloat16, tag="hT")
                for b in range(KB0):
                    nc.sync.dma_start(out=hT[:, b, :],
                                      in_=xT_in[:, b * NPAD:(b + 1) * NPAD])
                wl_t = w0pool.tile([128, KB0, 1024], dt.bfloat16)
                wr_t = w0pool.tile([128, KB0, 1024], dt.bfloat16)
                for wt, win in ((wl_t, w_in[0][0]), (wr_t, w_in[0][1])):
                    for b in range(KB0):
                        nc.sync.dma_start(
                            out=wt[:, b, :],
                            in_=win[:, b * 1024:(b + 1) * 1024])
                for q in range(NQ):
                    proj_quarter(0, q, hT, q * QROWS, wl_t, wr_t,
                                 psA0, mm0pool)

            for l, (K, F, H, concat) in enumerate(LAYERS):
                KB = K // 128
                C = NHID                # 256 channels per head
                if l < 2:
                    Kn, Fn = LAYERS[l + 1][0], LAYERS[l + 1][1]
                    KBn = Kn // 128

                # ---- edge phase (with next-layer proj interleaved) ----
                with (
                    tc.tile_pool(name=f"g{l}",
                                 bufs=3 if l == 0 else 2) as gpool,
                    tc.tile_pool(name=f"gx{l}", bufs=3) as gxpool,
                    tc.tile_pool(name=f"ew{l}", bufs=2) as epool,
                    tc.tile_pool(name=f"es{l}",
                                 bufs=3 if l == 0 else 2) as spool,
                    tc.tile_pool(name=f"psE{l}", bufs=1,
                                 space="PSUM") as psE,
                    tc.tile_pool(name=f"psD{l}", bufs=1,
                                 space="PSUM") as psD,
                    tc.tile_pool(name=f"psP{l}", bufs=1,
                                 space="PSUM") as psP,
                    tc.tile_pool(name=f"aux{l}", bufs=1) as auxpool,
                    tc.tile_pool(name=f"wn{l}", bufs=1) as wnpool,
                    tc.tile_pool(name=f"hTn{l}", bufs=2) as hnpool,
                    tc.tile_pool(name=f"mmn{l}", bufs=4) as mmnpool,
                    tc.tile_pool(name=f"psAn{l}", bufs=2,
                                 space="PSUM") as psAn,
                ):
                    att_t = auxpool.tile([128, F], dt.bfloat16)
                    nc.sync.dma_start(out=att_t[:, :], in_=att_in[l][:, :])
                    if l < 2:
                        wln = wnpool.tile([128, KBn, Fn], dt.bfloat16)
                        wrn = wnpool.tile([128, KBn, Fn], dt.bfloat16)
                        for wt, win in ((wln, w_in[l + 1][0]),
                                        (wrn, w_in[l + 1][1])):
                            for b in range(KBn):
                                nc.sync.dma_start(
                                    out=wt[:, b, :],
                                    in_=win[:, b * Fn:(b + 1) * Fn])
                    bias_t = auxpool.tile([128, F if concat else 256],
                                          dt.bfloat16 if concat else dt.float32)
                    nc.sync.dma_start(out=bias_t[:, :], in_=b_in[l][:, :])
                    if l == 2:
                        pmask_t = auxpool.tile([128, NWIN * N_GRAPHS],
                                               dt.bfloat16)
                        nc.sync.dma_start(out=pmask_t[:, :], in_=pmask_in[:, :])
                        pacc = auxpool.tile([128, 2, N_GRAPHS], dt.float32)

                    goff = 0   # running slot offset into igx chunk stream
                    for w in range(NWIN):
                        T = Tw[w]
                        S = T * 128
                        nchunks = (T + CT - 1) // CT

                        # per-window PSUM accumulators (one bank per head:
                        # start=True clears the whole bank, so accumulation
                        # chains must not share banks)
                        ps_agg = [psE.tile([128, C], dt.float32,
                                           tag=f"agg{j}", name=f"agg{j}")
                                  for j in range(H)]
                        ps_den = psD.tile([128, H], dt.float32, tag="den")

                        # per-window compact ex (den matmul rhs)
                        exb_w = spool.tile([128, TMAX, H], dt.bfloat16,
                                           tag="exbw")

                        mask_t = epool.tile([128, TMAX * 128], dt.bfloat16,
                                            tag="emask")
                        nc.sync.dma_start(
                            out=mask_t[:, :S],
                            in_=emask_in[:, Soff[w]:Soff[w + 1]])
                        # per-window gather of xr[dst] rows (local)
                        gr_w = gpool.tile([128, TMAX, F], dt.bfloat16,
                                          tag="gr")
                        nc.gpsimd.dma_gather(
                            gr_w[:, :T, :], xr_loc[l][:, :],
                            idst_t[:, Soff[w] // 16:Soff[w + 1] // 16],
                            num_idxs=S, num_idxs_reg=S, elem_size=F)

                        for ci in range(nchunks):
                            t0 = ci * CT
                            tcn = min(CT, T - t0)
                            Sc = tcn * 128

                            gx_c = gxpool.tile([128, CT, F], dt.bfloat16,
                                               tag="gx")
                            nc.gpsimd.dma_gather(
                                gx_c[:, :tcn, :], xl_full[l][:, :],
                                isrc_t[:, goff // 16:(goff + Sc) // 16],
                                num_idxs=Sc, num_idxs_reg=Sc,
                                elem_size=F)
                            goff += Sc
                            gx = gx_c[:, 0:tcn, :]

                            # s = gx + gr ; p = prelu(s) ; patt = p*att
                            s_t = spool.tile([128, CT, F], dt.bfloat16,
                                             tag="s")
                            nc.vector.tensor_tensor(
                                s_t[:, :tcn, :], gx,
                                gr_w[:, t0:t0 + tcn, :], OP.add)
                            nc.scalar.activation(
                                s_t[:, :tcn, :], s_t[:, :tcn, :],
                                AF.Prelu, alpha=SLOPE)
                            nc.vector.tensor_tensor(
                                s_t[:, :tcn, :], s_t[:, :tcn, :],
                                att_t[:, None, :]
                                    .to_broadcast((128, tcn, F)),
                                OP.mult)

                            # scores: sc[p, t, h] = sum_c patt
                            sc_t = spool.tile([128, CT, H], dt.float32,
                                              tag="sc")
                            nc.vector.tensor_reduce(
                                sc_t[:, :tcn, :],
                                s_t[:, :tcn, :].rearrange(
                                    "p t (h c) -> p t h c", h=H),
                                mybir.AxisListType.X, OP.add)

                            # ex replicated 128-wide via stride-0 exp input
                            exB = spool.tile([128, CT, H, 128], dt.bfloat16,
                                             tag="exB")
                            nc.scalar.activation(
                                exB[:, :tcn, :, :],
                                sc_t[:, :tcn, :][:, :, :, None]
                                    .to_broadcast((128, tcn, H, 128)),
                                AF.Exp)
                            # compact ex for the den matmul
                            nc.scalar.activation(
                                exb_w[:, t0:t0 + tcn, :],
                                sc_t[:, :tcn, :], AF.Exp)

                            # maskS[p,t,h,n] = mask[p,t,n] * exB[p,t,h,n]
                            maskS = spool.tile([128, CT, H, 128], dt.bfloat16,
                                               tag="maskS")
                            nc.vector.tensor_tensor(
                                maskS[:, :tcn, :, :],
                                mask_t[:, t0 * 128:(t0 + tcn) * 128]
                                    .rearrange("p (t n) -> p t n", t=tcn)
                                    [:, :, None, :]
                                    .to_broadcast((128, tcn, H, 128)),
                                exB[:, :tcn, :, :], OP.mult)

                            # den += mask^T @ ex ; agg[h] += maskS^T @ gx
                            for t in range(tcn):
                                tg = t0 + t
                                nc.tensor.matmul(
                                    ps_den[:, :],
                                    mask_t[:, tg * 128:(tg + 1) * 128],
                                    exb_w[:, tg, :],
                                    start=(tg == 0), stop=(tg == T - 1))
                            for t in range(tcn):
                                tg = t0 + t
                                for h in range(H):
                                    nc.tensor.matmul(
                                        ps_agg[h][:, :],
                                        maskS[:, t, h, :],
                                        gx[:, t, h * C:(h + 1) * C],
                                        start=(tg == 0), stop=(tg == T - 1))

                        # ---- window epilogue ----
                        den_t = spool.tile([128, H], dt.float32, tag="wden")
                        if concat:
                            nc.vector.tensor_scalar(
                                den_t[:, :], ps_den[:, :H], float(EPS),
                                None, OP.add)
                        else:
                            # fold the mean-over-heads 1/H into rec
                            nc.vector.tensor_scalar(
                                den_t[:, :], ps_den[:, :H], float(EPS),
                                float(H), OP.add, OP.mult)
                        rec_t = spool.tile([128, H], dt.float32, tag="wrec")
                        nc.vector.reciprocal(rec_t[:, :], den_t[:, :])

                        if DEBUG and l == 0 and w == 0:
                            dden = spool.tile([128, H], dt.float32,
                                              tag="dden")
                            nc.vector.tensor_copy(dden[:, :], den_t[:, :])
                            nc.sync.dma_start(out=dbg_den[:, :H],
                                              in_=dden[:, :])
                        if concat:
                            # z = agg*rec + bias (bf16), h = elu(z)
                            z_t = spool.tile([128, F], dt.bfloat16, tag="z")
                            for h in range(H):
                                nc.vector.tensor_scalar(
                                    z_t[:, h * C:(h + 1) * C],
                                    ps_agg[h][:, :], rec_t[:, h:h + 1],
                                    None, OP.mult)
                            if DEBUG and l == 0 and w == 0:
                                dz = spool.tile([128, F], dt.float32,
                                                tag="dz")
                                nc.vector.tensor_copy(dz[:, :], z_t[:, :])
                                nc.sync.dma_start(out=dbg_z[:, :F],
                                                  in_=dz[:, :])
                            nc.vector.tensor_tensor(
                                z_t[:, :], z_t[:, :], bias_t[:, :], OP.add)
                            e_t = spool.tile([128, F], dt.bfloat16,
                                             tag="elu_e")
                            nc.scalar.activation(e_t[:, :], z_t[:, :], AF.Exp)
                            # m = min(e,1) - 1
                            nc.vector.tensor_scalar(
                                e_t[:, :], e_t[:, :], 1.0, -1.0,
                                OP.min, OP.add)
                            hb = spool.tile([128, F], dt.bfloat16, tag="hb")
                            nc.vector.tensor_tensor(hb[:, :], z_t[:, :],
                                                    e_t[:, :], OP.max)
                            nc.sync.dma_start(
                                out=h_dram[l][w * 128:(w + 1) * 128, :],
                                in_=hb[:, :])
                        else:
                            # mean over heads (1/H folded into rec), + bias
                            acc = spool.tile([128, 256], dt.float32, tag="acc")
                            nc.vector.tensor_scalar(
                                acc[:, :], ps_agg[0][:, :], rec_t[:, 0:1],
                                None, OP.mult)
                            for h in range(1, H):
                                nc.vector.scalar_tensor_tensor(
                                    acc[:, :], ps_agg[h][:, :],
                                    rec_t[:, h:h + 1], acc[:, :],
                                    OP.mult, OP.add)
                            nc.vector.tensor_tensor(acc[:, :], acc[:, :],
                                                    bias_t[:, :], OP.add)
                            # l2 normalize rows
                            ss = spool.tile([128, 1], dt.float32, tag="ss")
                            trash2 = spool.tile([128, 256], dt.float32,
                                                tag="trash2")
                            nc.vector.scalar_tensor_tensor(
                                trash2[:, :], acc[:, :], 1.0, acc[:, :],
                                OP.mult, OP.mult, accum_out=ss[:, :])
                            nrm = spool.tile([128, 1], dt.float32, tag="nrm")
                            nc.scalar.activation(nrm[:, :], ss[:, :], AF.Sqrt)
                            nc.vector.tensor_scalar(nrm[:, :], nrm[:, :],
                                                    1e-12, None, OP.max)
                            rn = spool.tile([128, 1], dt.float32, tag="rn")
                            nc.vector.reciprocal(rn[:, :], nrm[:, :])
                            hb = spool.tile([128, 256], dt.bfloat16,
                                            tag="hb")
                            nc.vector.tensor_scalar(hb[:, :], acc[:, :],
                                                    rn[:, :], None, OP.mult)
                            # pool: pooled_T[c, g] += sum_n h[n, c] pmask[n, g]
                            # (one scratch bank; accumulate in SBUF pacc)
                            ps_pool = psP.tile([128, N_GRAPHS], dt.float32,
                                               tag="poolmm")
                            for b in range(2):
                                nc.tensor.matmul(
                                    ps_pool[:, :],
                                    hb[:, b * 128:(b + 1) * 128],
                                    pmask_t[:, w * N_GRAPHS:
                                            (w + 1) * N_GRAPHS],
                                    start=True, stop=True)
                                if w == 0:
                                    nc.vector.tensor_copy(pacc[:, b, :],
                                                          ps_pool[:, :])
                                else:
                                    nc.vector.tensor_tensor(
                                        pacc[:, b, :], pacc[:, b, :],
                                        ps_pool[:, :], OP.add)

                        # every NWIN/NQ windows: project the finished quarter
                        # of the NEXT layer and AllGather its xl chunk
                        if l < 2 and (w + 1) % (NWIN // NQ) == 0:
                            q = w // (NWIN // NQ)
                            hTq = hnpool.tile([128, KBn, QROWS], dt.bfloat16,
                                              tag="hTq")
                            for b in range(KBn):
                                nc.sync.dma_start(
                                    out=hTq[:, b, :],
                                    in_=h_dram[l][q * QROWS:(q + 1) * QROWS,
                                                  b * 128:(b + 1) * 128],
                                    transpose=True)
                            proj_quarter(l + 1, q, hTq, 0, wln, wrn,
                                         psAn, mmnpool)

                    if l == 2:
                        for b in range(2):
                            nc.sync.dma_start(
                                out=pool_loc[b * 128:(b + 1) * 128, :],
                                in_=pacc[:, b, :])

            if DEBUG:
                nc.sync.dma_start(out=dbg_xg0[0:GR_BASE, :],
                                  in_=xl_full[0][:, :])
                nc.sync.dma_start(out=dbg_xg0[GR_BASE:GR_BASE + NPAD, :],
                                  in_=xr_loc[0][:, :])
                nc.sync.dma_start(out=dbg_h0[:, :], in_=h_dram[0][:, :])
                nc.sync.dma_start(out=dbg_h1[:, :], in_=h_dram[1][:, :])

            # ---- pooled -> AllReduce -> MLP ----
            with (
                tc.tile_pool(name="mlp", bufs=1) as mpool,
                tc.tile_pool(name="psM", bufs=1, space="PSUM") as psM,
            ):
                nc.gpsimd.collective_compute(
                    "AllReduce", mybir.AluOpType.add, replica_groups=rg,
                    ins=[pool_loc.ap().opt()],
                    outs=[pool_full.ap().opt()])
                if DEBUG:
                    nc.sync.dma_start(out=dbg_pool[:, :], in_=pool_full[:, :])

                rcnt_t = mpool.tile([128, N_GRAPHS], dt.float32)
                nc.sync.dma_start(out=rcnt_t[:, :], in_=rcnt_in[:, :])
                pz = mpool.tile([128, 2, N_GRAPHS], dt.bfloat16)
                for b in range(2):
                    pf = mpool.tile([128, N_GRAPHS], dt.float32, tag="pf")
                    nc.sync.dma_start(out=pf[:, :],
                                      in_=pool_full[b * 128:(b + 1) * 128, :])
                    nc.vector.tensor_tensor(pz[:, b, :], pf[:, :],
                                            rcnt_t[:, :], OP.mult)

                wfc1_t = mpool.tile([128, 2, 256], dt.bfloat16)
                wfc2_t = mpool.tile([128, 2, 768], dt.bfloat16)
                for b in range(2):
                    nc.sync.dma_start(out=wfc1_t[:, b, :],
                                      in_=wfc1_in[:, b * 256:(b + 1) * 256])
                    nc.sync.dma_start(out=wfc2_t[:, b, :],
                                      in_=wfc2_in[:, b * 768:(b + 1) * 768])
                bfc1_t = mpool.tile([128, 2], dt.float32)
                nc.sync.dma_start(out=bfc1_t[:, :], in_=bfc1_in[:, :])
                bfc2_t = mpool.tile([128, 768], dt.float32)
                nc.sync.dma_start(out=bfc2_t[:, :], in_=bfc2_in[:, :])

                z1 = mpool.tile([128, 2, N_GRAPHS], dt.bfloat16)
                for it in range(2):
                    ps1 = psM.tile([128, N_GRAPHS], dt.float32, tag="ps1")
                    for b in range(2):
                        nc.tensor.matmul(
                            ps1[:, :],
                            wfc1_t[:, b, it * 128:(it + 1) * 128],
                            pz[:, b, :], start=(b == 0), stop=(b == 1))
                    nc.scalar.activation(z1[:, it, :], ps1[:, :], AF.Relu,
                                         bias=bfc1_t[:, it:it + 1], scale=1.0)

                for gt in range(N_GRAPHS // 128):
                    ps2 = psM.tile([128, 768], dt.float32, tag="ps2")
                    for jc, (j0, jw) in enumerate(((0, 512), (512, 256))):
                        for b in range(2):
                            nc.tensor.matmul(
                                ps2[:, j0:j0 + jw],
                                z1[:, b, gt * 128:(gt + 1) * 128],
                                wfc2_t[:, b, j0:j0 + jw],
                                start=(b == 0), stop=(b == 1))
                    zo = mpool.tile([128, 768], dt.float32, tag="zo")
                    nc.vector.tensor_tensor(zo[:, :], ps2[:, :],
                                            bfc2_t[:, :], OP.add)
                    nc.sync.dma_start(
                        out=out_ext[gt * 128:(gt + 1) * 128, :], in_=zo[:, :])

    nc.compile()
    return nc


def kernel(**inputs):
    T, in_maps = _preprocess(inputs)
    if T not in _PROG_CACHE:
        _PROG_CACHE[T] = _build(T)
    nc = _PROG_CACHE[T]
    r = run_bass_kernel_spmd(nc, in_maps, list(range(NCORES)), trace=False)
    return r.results[0]["out"]


# revision 48
# speedup vs baseline: 1.1058x; 1.1058x over previous
"""GATv2 (3-layer, PyG semantics) + global mean pool + MLP on 8 trn2 NeuronCores.

Self-contained: hardcodes problem shapes from nn_GAT_47906065219807.
Sharding: data-parallel over contiguous node ranges (2500 nodes/core); each
core owns edges whose dst lands in its range (edges cross shards via an
AllGather of the source-side projections).

v2 edge phase (per 128-dst-node window, chunks of <=4 edge tiles):
  - ONE merged dma_gather per chunk pulls both xl[src] and xr[dst] rows from a
    unified DRAM buffer (AllGathered xl ++ locally-written xr).
  - s = gx + gr (one batched TT), p = prelu(s) (one batched ACT op)
  - score[e,h] = sum_c att[h,c] * p[e,h,c] via scalar_tensor_tensor accum_out
    (fuses the att multiply and the segmented reduce)
  - exp with a stride-0-broadcast input writes ex replicated 128-wide, so the
    per-head mask scaling is ONE 2x-mode TT per chunk
  - den via per-tile matmul (binary mask x ex), aggregation via per-(tile,head)
    matmul (scaled mask x raw gx) accumulated in PSUM
"""
import numpy as np
import ml_dtypes

import concourse.bacc as bacc
import concourse.mybir as mybir
import concourse.tile as tile
from concourse.bass_utils import run_bass_kernel_spmd

# problem constants
N_NODES = 20000
N_EDGES = 120000
N_GRAPHS = 512
F_IN = 300
NHID = 256
NOUT = 768
SLOPE = 0.2
EPS = 1e-16

NCORES = 8
NLOC = N_NODES // NCORES            # 2500
NPAD = 2560                         # 20 tiles of 128
NWIN = NPAD // 128                  # 20 windows / node tiles per core
KIN_PAD = 384                       # F_IN padded to 3*128
GR_BASE = NCORES * NPAD             # total rows in xl_full
CT = 4                              # edge tiles per gather chunk
NQ = 4                              # projection/AllGather chunks per layer
QROWS = NPAD // NQ                  # 640 rows per chunk
QTILES = QROWS // 128               # 5 node tiles per chunk

# per-layer dims: (K_in_padded, F_out, heads, concat)
LAYERS = [
    (KIN_PAD, 4 * NHID, 4, True),
    (4 * NHID, 4 * NHID, 4, True),
    (4 * NHID, 6 * NHID, 6, False),
]

_BF = ml_dtypes.bfloat16
_PROG_CACHE = {}
DEBUG = False


def _bf16(a):
    return np.ascontiguousarray(a.astype(_BF)).view(np.uint16)


def _wrap_idx(flat_idx):
    """dma_gather index layout: slot i -> [i % 16, i // 16], replicated to
    128 partitions (8 Q7 cores x 16)."""
    n = flat_idx.shape[0]
    assert n % 16 == 0
    w = flat_idx.reshape(n // 16, 16).T.astype(np.int16)
    return np.tile(w, (8, 1)).copy()


def _preprocess(inputs):
    """Host-side sharding/layout. Returns (Tw, in_maps)."""
    x = np.asarray(inputs["x"], np.float32)
    ei = np.asarray(inputs["edge_index"]).astype(np.int64)
    batch = np.asarray(inputs["batch"]).astype(np.int64)

    loops = np.arange(N_NODES, dtype=np.int64)
    src = np.concatenate([ei[0], loops])
    dst = np.concatenate([ei[1], loops])

    # row index into xl_full, whose layout is chunk-major:
    # [quarter q][core c][row r within quarter]  (AllGather chunk per quarter)
    s_core = src // NLOC
    s_loc = src % NLOC
    s_q = s_loc // QROWS
    src_pad = s_q * (NCORES * QROWS) + s_core * QROWS + (s_loc - s_q * QROWS)

    core_of = dst // NLOC
    buckets = [[[] for _ in range(NWIN)] for _ in range(NCORES)]
    order = np.argsort(dst, kind="stable")
    for e in order:
        c = core_of[e]
        dl = dst[e] - c * NLOC
        buckets[c][dl // 128].append(e)
    Tw = tuple(
        max((len(buckets[c][w]) + 127) // 128 for c in range(NCORES))
        for w in range(NWIN)
    )
    Soff = np.concatenate([[0], np.cumsum([t * 128 for t in Tw])])
    TOT = int(Soff[-1])

    # graph counts -> reciprocal (reference divides by max(cnt,1))
    cnt = np.bincount(batch, minlength=N_GRAPHS).astype(np.float32)
    rcnt = 1.0 / np.maximum(cnt, 1.0)

    def wT_pad(w, kpad):
        # host layout [128, KB, F]: [p, b, f]; w is [F, K]
        wt = w.T.astype(np.float32)                    # [K, F]
        K, F = wt.shape
        out = np.zeros((kpad, F), np.float32)
        out[:K] = wt
        return _bf16(out.reshape(kpad // 128, 128, F).transpose(1, 0, 2)
                     .reshape(128, (kpad // 128) * F))

    shared = {
        "w1l": wT_pad(np.asarray(inputs["c1_wl"]), KIN_PAD),
        "w1r": wT_pad(np.asarray(inputs["c1_wr"]), KIN_PAD),
        "w2l": wT_pad(np.asarray(inputs["c2_wl"]), 4 * NHID),
        "w2r": wT_pad(np.asarray(inputs["c2_wr"]), 4 * NHID),
        "w3l": wT_pad(np.asarray(inputs["c3_wl"]), 4 * NHID),
        "w3r": wT_pad(np.asarray(inputs["c3_wr"]), 4 * NHID),
        "att1": _bf16(np.tile(np.asarray(inputs["c1_att"]).reshape(1, -1), (128, 1))),
        "att2": _bf16(np.tile(np.asarray(inputs["c2_att"]).reshape(1, -1), (128, 1))),
        "att3": _bf16(np.tile(np.asarray(inputs["c3_att"]).reshape(1, -1), (128, 1))),
        "b1": _bf16(np.tile(np.asarray(inputs["c1_b"], np.float32).reshape(1, -1), (128, 1))),
        "b2": _bf16(np.tile(np.asarray(inputs["c2_b"], np.float32).reshape(1, -1), (128, 1))),
        "b3": np.tile(np.asarray(inputs["c3_b"], np.float32).reshape(1, -1), (128, 1)),
        "rcnt": np.tile(rcnt.reshape(1, -1), (128, 1)).astype(np.float32),
        "wfc1": wT_pad(np.asarray(inputs["fp1_w"]), 256),
        "wfc2": wT_pad(np.asarray(inputs["fp2_w"]), 256),
        "bfc1": np.asarray(inputs["fp1_b"], np.float32).reshape(2, 128).T.copy(),
        "bfc2": np.tile(np.asarray(inputs["fp2_b"], np.float32).reshape(1, -1),
                        (128, 1)),
    }

    in_maps = []
    for c in range(NCORES):
        xc = np.zeros((NPAD, KIN_PAD), np.float32)
        xc[:NLOC, :F_IN] = x[c * NLOC:(c + 1) * NLOC]
        xT = xc.T.reshape(KIN_PAD // 128, 128, NPAD).transpose(1, 0, 2)
        xT = _bf16(xT.reshape(128, (KIN_PAD // 128) * NPAD))

        # gather indices: window-contiguous src rows (into xl_full) and
        # dst rows (into xr_loc)
        isrc = np.zeros(TOT, np.int64)
        idst = np.zeros(TOT, np.int64)
        emask = np.zeros((128, TOT), np.float32)
        for w in range(NWIN):
            es = buckets[c][w]
            s0 = int(Soff[w])
            # emask: slot (tile t, partition p) with dst row n ->
            # emask[p, s0 + t*128 + n] = 1
            for i, e in enumerate(es):
                n = (dst[e] - c * NLOC) - w * 128
                emask[i % 128, s0 + (i // 128) * 128 + n] = 1.0
                isrc[s0 + i] = src_pad[e]
                idst[s0 + i] = dst[e] - c * NLOC

        pmask = np.zeros((128, NWIN * N_GRAPHS), np.float32)
        bl = batch[c * NLOC:(c + 1) * NLOC]
        for nl in range(NLOC):
            pmask[nl % 128, (nl // 128) * N_GRAPHS + bl[nl]] = 1.0

        m = dict(shared)
        m["xT"] = xT
        m["isrc"] = _wrap_idx(isrc)
        m["idst"] = _wrap_idx(idst)
        m["emask"] = _bf16(emask)
        m["pmask"] = _bf16(pmask)
        in_maps.append(m)
    return Tw, in_maps


def _build(Tw):
    Tw = tuple(Tw)
    TMAX = max(Tw)
    Soff = [0]
    for t in Tw:
        Soff.append(Soff[-1] + t * 128)
    TOT = Soff[-1]
    nc = bacc.Bacc("TRN2", target_bir_lowering=False, debug=False,
                   num_devices=NCORES)
    dt = mybir.dt
    AF = mybir.ActivationFunctionType
    OP = mybir.AluOpType

    def inp(name, shape, d):
        return nc.dram_tensor(name, shape, d, kind="ExternalInput")

    xT_in = inp("xT", [128, (KIN_PAD // 128) * NPAD], dt.bfloat16)
    isrc_in = inp("isrc", [128, TOT // 16], dt.int16)
    idst_in = inp("idst", [128, TOT // 16], dt.int16)
    emask_in = inp("emask", [128, TOT], dt.bfloat16)
    pmask_in = inp("pmask", [128, NWIN * N_GRAPHS], dt.bfloat16)
    w_in = [(inp("w1l", [128, 3 * 1024], dt.bfloat16),
             inp("w1r", [128, 3 * 1024], dt.bfloat16)),
            (inp("w2l", [128, 8 * 1024], dt.bfloat16),
             inp("w2r", [128, 8 * 1024], dt.bfloat16)),
            (inp("w3l", [128, 8 * 1536], dt.bfloat16),
             inp("w3r", [128, 8 * 1536], dt.bfloat16))]
    att_in = [inp("att1", [128, 1024], dt.bfloat16),
              inp("att2", [128, 1024], dt.bfloat16),
              inp("att3", [128, 1536], dt.bfloat16)]
    b_in = [inp("b1", [128, 1024], dt.bfloat16),
            inp("b2", [128, 1024], dt.bfloat16),
            inp("b3", [128, 256], dt.float32)]
    rcnt_in = inp("rcnt", [128, N_GRAPHS], dt.float32)
    wfc1_in = inp("wfc1", [128, 2 * 256], dt.bfloat16)
    wfc2_in = inp("wfc2", [128, 2 * 768], dt.bfloat16)
    bfc1_in = inp("bfc1", [128, 2], dt.float32)
    bfc2_in = inp("bfc2", [128, 768], dt.float32)
    out_ext = nc.dram_tensor("out", [N_GRAPHS, NOUT], dt.float32,
                             kind="ExternalOutput")
    if DEBUG:
        dbg_xg0 = nc.dram_tensor("dbg_xg0", [GR_BASE + NPAD, 1024],
                                 dt.bfloat16, kind="ExternalOutput")
        dbg_h0 = nc.dram_tensor("dbg_h0", [NPAD, 1024], dt.bfloat16,
                                kind="ExternalOutput")
        dbg_h1 = nc.dram_tensor("dbg_h1", [NPAD, 1024], dt.bfloat16,
                                kind="ExternalOutput")
        dbg_pool = nc.dram_tensor("dbg_pool", [256, N_GRAPHS], dt.float32,
                                  kind="ExternalOutput")
        dbg_den = nc.dram_tensor("dbg_den", [128, 6], dt.float32,
                                 kind="ExternalOutput")
        dbg_z = nc.dram_tensor("dbg_z", [128, 1024], dt.float32,
                               kind="ExternalOutput")

    # internal DRAM
    xl_loc = [nc.dram_tensor(f"xl_loc{l}", [NPAD, F], dt.bfloat16)
              for l, (_, F, _, _) in enumerate(LAYERS)]
    xr_loc = [nc.dram_tensor(f"xr_loc{l}", [NPAD, F], dt.bfloat16)
              for l, (_, F, _, _) in enumerate(LAYERS)]
    xl_full = [nc.dram_tensor(f"xl_full{l}", [GR_BASE, F], dt.bfloat16,
                              addr_space="Shared")
               for l, (_, F, _, _) in enumerate(LAYERS)]
    h_dram = [nc.dram_tensor(f"h_dram{l}", [NPAD, 1024], dt.bfloat16)
              for l in range(2)]
    pool_loc = nc.dram_tensor("pool_loc", [256, N_GRAPHS], dt.float32)
    pool_full = nc.dram_tensor("pool_full", [256, N_GRAPHS], dt.float32,
                               addr_space="Shared")

    rg = [list(range(NCORES))]

    with tile.TileContext(nc) as tc:
        with (
            tc.tile_pool(name="persist", bufs=1) as ppool,
        ):
            isrc_t = ppool.tile([128, TOT // 16], dt.int16)
            nc.sync.dma_start(out=isrc_t[:, :], in_=isrc_in[:, :])
            idst_t = ppool.tile([128, TOT // 16], dt.int16)
            nc.sync.dma_start(out=idst_t[:, :], in_=idst_in[:, :])

            def proj_quarter(l, q, hT_q, hoff, wl_t, wr_t, psA, mmpool):
                """Project rows [q*QROWS, (q+1)*QROWS) of layer l's input;
                issue the AllGather chunk for the xl side."""
                K, F, _, _ = LAYERS[l]
                KB = K // 128
                NCH = F // 512
                for side, (wt, dst_dram) in enumerate(
                        ((wl_t, xl_loc[l]), (wr_t, xr_loc[l]))):
                    for tq in range(QTILES):
                        t = q * QTILES + tq
                        for ch in range(NCH):
                            ps = psA.tile([128, 512], dt.float32, tag="mmps")
                            for b in range(KB):
                                nc.tensor.matmul(
                                    ps[:, :],
                                    hT_q[:, b, hoff + tq * 128:
                                         hoff + (tq + 1) * 128],
                                    wt[:, b, ch * 512:(ch + 1) * 512],
                                    start=(b == 0), stop=(b == KB - 1))
                            ob = mmpool.tile([128, 512], dt.bfloat16,
                                             tag="mmout")
                            nc.scalar.copy(ob[:, :], ps[:, :])
                            nc.sync.dma_start(
                                out=dst_dram[t * 128:(t + 1) * 128,
                                             ch * 512:(ch + 1) * 512],
                                in_=ob[:, :])
                    if side == 0:
                        nc.gpsimd.collective_compute(
                            "AllGather", mybir.AluOpType.bypass,
                            replica_groups=rg,
                            ins=[xl_loc[l][q * QROWS:
                                           (q + 1) * QROWS, :].opt()],
                            outs=[xl_full[l][q * NCORES * QROWS:
                                             (q + 1) * NCORES *
                                             QROWS, :].opt()])

            # ---- initial projection: layer 0 (quartered, AG overlapped) ----
            with (
                tc.tile_pool(name="w0", bufs=1) as w0pool,
                tc.tile_pool(name="hT0", bufs=1) as h0pool,
                tc.tile_pool(name="mm0", bufs=4) as mm0pool,
                tc.tile_pool(name="psA0", bufs=2, space="PSUM") as psA0,
            ):
                KB0 = KIN_PAD // 128
                hT = h0pool.tile([128, KB0, NPAD], dt.bfloat16, tag="hT")
                for b in range(KB0):
                    nc.sync.dma_start(out=hT[:, b, :],
                                      in_=xT_in[:, b * NPAD:(b + 1) * NPAD])
                wl_t = w0pool.tile([128, KB0, 1024], dt.bfloat16)
                wr_t = w0pool.tile([128, KB0, 1024], dt.bfloat16)
                for wt, win in ((wl_t, w_in[0][0]), (wr_t, w_in[0][1])):
                    for b in range(KB0):
                        nc.sync.dma_start(
                            out=wt[:, b, :],
                            in_=win[:, b * 1024:(b + 1) * 1024])
                for q in range(NQ):
                    proj_quarter(0, q, hT, q * QROWS, wl_t, wr_t,
                                 psA0, mm0pool)

            for l, (K, F, H, concat) in enumerate(LAYERS):
                KB = K // 128
                C = NHID                # 256 channels per head
                if l < 2:
                    Kn, Fn = LAYERS[l + 1][0], LAYERS[l + 1][1]
                    KBn = Kn // 128

                # ---- edge phase (with next-layer proj interleaved) ----
                with (
                    tc.tile_pool(name=f"g{l}", bufs=2) as gpool,
                    tc.tile_pool(name=f"gx{l}", bufs=3) as gxpool,
                    tc.tile_pool(name=f"ew{l}", bufs=2) as epool,
                    tc.tile_pool(name=f"es{l}", bufs=2) as spool,
                    tc.tile_pool(name=f"psE{l}", bufs=1,
                                 space="PSUM") as psE,
                    tc.tile_pool(name=f"psD{l}", bufs=1,
                                 space="PSUM") as psD,
                    tc.tile_pool(name=f"psP{l}", bufs=1,
                                 space="PSUM") as psP,
                    tc.tile_pool(name=f"aux{l}", bufs=1) as auxpool,
                    tc.tile_pool(name=f"wn{l}", bufs=1) as wnpool,
                    tc.tile_pool(name=f"hTn{l}", bufs=2) as hnpool,
                    tc.tile_pool(name=f"mmn{l}", bufs=4) as mmnpool,
                    tc.tile_pool(name=f"psAn{l}", bufs=2,
                                 space="PSUM") as psAn,
                ):
                    att_t = auxpool.tile([128, F], dt.bfloat16)
                    nc.sync.dma_start(out=att_t[:, :], in_=att_in[l][:, :])
                    if l < 2:
                        wln = wnpool.tile([128, KBn, Fn], dt.bfloat16)
                        wrn = wnpool.tile([128, KBn, Fn], dt.bfloat16)
                        for wt, win in ((wln, w_in[l + 1][0]),
                                        (wrn, w_in[l + 1][1])):
                            for b in range(KBn):
                                nc.sync.dma_start(
                                    out=wt[:, b, :],
                                    in_=win[:, b * Fn:(b + 1) * Fn])
                    bias_t = auxpool.tile([128, F if concat else 256],
                                          dt.bfloat16 if concat else dt.float32)
                    nc.sync.dma_start(out=bias_t[:, :], in_=b_in[l][:, :])
                    if l == 2:
                        pmask_t = auxpool.tile([128, NWIN * N_GRAPHS],
                                               dt.bfloat16)
                        nc.sync.dma_start(out=pmask_t[:, :], in_=pmask_in[:, :])
                        pacc = auxpool.tile([128, 2, N_GRAPHS], dt.float32)

                    goff = 0   # running slot offset into igx chunk stream
                    for w in range(NWIN):
                        T = Tw[w]
                        S = T * 128
                        nchunks = (T + CT - 1) // CT

                        # per-window PSUM accumulators (one bank per head:
                        # start=True clears the whole bank, so accumulation
                        # chains must not share banks)
                        ps_agg = [psE.tile([128, C], dt.float32,
                                           tag=f"agg{j}", name=f"agg{j}")
                                  for j in range(H)]
                        ps_den = psD.tile([128, H], dt.float32, tag="den")

                        # per-window compact ex (den matmul rhs)
                        exb_w = spool.tile([128, TMAX, H], dt.bfloat16,
                                           tag="exbw")

                        mask_t = epool.tile([128, TMAX * 128], dt.bfloat16,
                                            tag="emask")
                        nc.sync.dma_start(
                            out=mask_t[:, :S],
                            in_=emask_in[:, Soff[w]:Soff[w + 1]])
                        # per-window gather of xr[dst] rows (local)
                        gr_w = gpool.tile([128, TMAX, F], dt.bfloat16,
                                          tag="gr")
                        nc.gpsimd.dma_gather(
                            gr_w[:, :T, :], xr_loc[l][:, :],
                            idst_t[:, Soff[w] // 16:Soff[w + 1] // 16],
                            num_idxs=S, num_idxs_reg=S, elem_size=F)

                        for ci in range(nchunks):
                            t0 = ci * CT
                            tcn = min(CT, T - t0)
                            Sc = tcn * 128

                            gx_c = gxpool.tile([128, CT, F], dt.bfloat16,
                                               tag="gx")
                            nc.gpsimd.dma_gather(
                                gx_c[:, :tcn, :], xl_full[l][:, :],
                                isrc_t[:, goff // 16:(goff + Sc) // 16],
                                num_idxs=Sc, num_idxs_reg=Sc,
                                elem_size=F)
                            goff += Sc
                            gx = gx_c[:, 0:tcn, :]

                            # s = gx + gr ; p = prelu(s) ; patt = p*att
                            s_t = spool.tile([128, CT, F], dt.bfloat16,
                                             tag="s")
                            nc.vector.tensor_tensor(
                                s_t[:, :tcn, :], gx,
                                gr_w[:, t0:t0 + tcn, :], OP.add)
                            nc.scalar.activation(
                                s_t[:, :tcn, :], s_t[:, :tcn, :],
                                AF.Prelu, alpha=SLOPE)
                            nc.vector.tensor_tensor(
                                s_t[:, :tcn, :], s_t[:, :tcn, :],
                                att_t[:, None, :]
                                    .to_broadcast((128, tcn, F)),
                                OP.mult)

                            # scores: sc[p, t, h] = sum_c patt
                            sc_t = spool.tile([128, CT, H], dt.float32,
                                              tag="sc")
                            nc.vector.tensor_reduce(
                                sc_t[:, :tcn, :],
                                s_t[:, :tcn, :].rearrange(
                                    "p t (h c) -> p t h c", h=H),
                                mybir.AxisListType.X, OP.add)

                            # ex replicated 128-wide via stride-0 exp input
                            exB = spool.tile([128, CT, H, 128], dt.bfloat16,
                                             tag="exB")
                            nc.scalar.activation(
                                exB[:, :tcn, :, :],
                                sc_t[:, :tcn, :][:, :, :, None]
                                    .to_broadcast((128, tcn, H, 128)),
                                AF.Exp)
                            # compact ex for the den matmul
                            nc.scalar.activation(
                                exb_w[:, t0:t0 + tcn, :],
                                sc_t[:, :tcn, :], AF.Exp)

                            # maskS[p,t,h,n] = mask[p,t,n] * exB[p,t,h,n]
                            maskS = spool.tile([128, CT, H, 128], dt.bfloat16,
                                               tag="maskS")
                            nc.vector.tensor_tensor(
                                maskS[:, :tcn, :, :],
                                mask_t[:, t0 * 128:(t0 + tcn) * 128]
                                    .rearrange("p (t n) -> p t n", t=tcn)
                                    [:, :, None, :]
                                    .to_broadcast((128, tcn, H, 128)),
                                exB[:, :tcn, :, :], OP.mult)

                            # den += mask^T @ ex ; agg[h] += maskS^T @ gx
                            for t in range(tcn):
                                tg = t0 + t
                                nc.tensor.matmul(
                                    ps_den[:, :],
                                    mask_t[:, tg * 128:(tg + 1) * 128],
                                    exb_w[:, tg, :],
                                    start=(tg == 0), stop=(tg == T - 1))
                            for t in range(tcn):
                                tg = t0 + t
                                for h in range(H):
                                    nc.tensor.matmul(
                                        ps_agg[h][:, :],
                                        maskS[:, t, h, :],
                                        gx[:, t, h * C:(h + 1) * C],
                                        start=(tg == 0), stop=(tg == T - 1))

                        # ---- window epilogue ----
                        den_t = spool.tile([128, H], dt.float32, tag="wden")
                        if concat:
                            nc.vector.tensor_scalar(
                                den_t[:, :], ps_den[:, :H], float(EPS),
                                None, OP.add)
                        else:
                            # fold the mean-over-heads 1/H into rec
                            nc.vector.tensor_scalar(
                                den_t[:, :], ps_den[:, :H], float(EPS),
                                float(H), OP.add, OP.mult)
                        rec_t = spool.tile([128, H], dt.float32, tag="wrec")
                        nc.vector.reciprocal(rec_t[:, :], den_t[:, :])

                        if DEBUG and l == 0 and w == 0:
                            dden = spool.tile([128, H], dt.float32,
                                              tag="dden")
                            nc.vector.tensor_copy(dden[:, :], den_t[:, :])
                            nc.sync.dma_start(out=dbg_den[:, :H],
                                              in_=dden[:, :])
                        if concat:
                            # z = agg*rec + bias (bf16), h = elu(z)
                            z_t = spool.tile([128, F], dt.bfloat16, tag="z")
                            for h in range(H):
                                nc.vector.tensor_scalar(
                                    z_t[:, h * C:(h + 1) * C],
                                    ps_agg[h][:, :], rec_t[:, h:h + 1],
                                    None, OP.mult)
                            if DEBUG and l == 0 and w == 0:
                                dz = spool.tile([128, F], dt.float32,
                                                tag="dz")
                                nc.vector.tensor_copy(dz[:, :], z_t[:, :])
                                nc.sync.dma_start(out=dbg_z[:, :F],
                                                  in_=dz[:, :])
                            nc.vector.tensor_tensor(
                                z_t[:, :], z_t[:, :], bias_t[:, :], OP.add)
                            e_t = spool.tile([128, F], dt.bfloat16,
                                             tag="elu_e")
                            nc.scalar.activation(e_t[:, :], z_t[:, :], AF.Exp)
                            # m = min(e,1) - 1
                            nc.vector.tensor_scalar(
                                e_t[:, :], e_t[:, :], 1.0, -1.0,
                                OP.min, OP.add)
                            hb = spool.tile([128, F], dt.bfloat16, tag="hb")
                            nc.vector.tensor_tensor(hb[:, :], z_t[:, :],
                                                    e_t[:, :], OP.max)
                            nc.sync.dma_start(
                                out=h_dram[l][w * 128:(w + 1) * 128, :],
                                in_=hb[:, :])
                        else:
                            # mean over heads (1/H folded into rec), + bias
                            acc = spool.tile([128, 256], dt.float32, tag="acc")
                            nc.vector.tensor_scalar(
                                acc[:, :], ps_agg[0][:, :], rec_t[:, 0:1],
                                None, OP.mult)
                            for h in range(1, H):
                                nc.vector.scalar_tensor_tensor(
                                    acc[:, :], ps_agg[h][:, :],
                                    rec_t[:, h:h + 1], acc[:, :],
                                    OP.mult, OP.add)
                            nc.vector.tensor_tensor(acc[:, :], acc[:, :],
                                                    bias_t[:, :], OP.add)
                            # l2 normalize rows
                            ss = spool.tile([128, 1], dt.float32, tag="ss")
                            trash2 = spool.tile([128, 256], dt.float32,
                                                tag="trash2")
                            nc.vector.scalar_tensor_tensor(
                                trash2[:, :], acc[:, :], 1.0, acc[:, :],
                                OP.mult, OP.mult, accum_out=ss[:, :])
                            nrm = spool.tile([128, 1], dt.float32, tag="nrm")
                            nc.scalar.activation(nrm[:, :], ss[:, :], AF.Sqrt)
                            nc.vector.tensor_scalar(nrm[:, :], nrm[:, :],
                                                    1e-12, None, OP.max)
                            rn = spool.tile([128, 1], dt.float32, tag="rn")
                            nc.vector.reciprocal(rn[:, :], nrm[:, :])
                            hb = spool.tile([128, 256], dt.bfloat16,
                                            tag="hb")
                            nc.vector.tensor_scalar(hb[:, :], acc[:, :],
                                                    rn[:, :], None, OP.mult)
                            # pool: pooled_T[c, g] += sum_n h[n, c] pmask[n, g]
                            # (one scratch bank; accumulate in SBUF pacc)
                            ps_pool = psP.tile([128, N_GRAPHS], dt.float32,
                                               tag="poolmm")
                            for b in range(2):
                                nc.tensor.matmul(
                                    ps_pool[:, :],
                                    hb[:, b * 128:(b + 1) * 128],
                                    pmask_t[:, w * N_GRAPHS:
                                            (w + 1) * N_GRAPHS],
                                    start=True, stop=True)
                                if w == 0:
                                    nc.vector.tensor_copy(pacc[:, b, :],
                                                          ps_pool[:, :])
                                else:
                                    nc.vector.tensor_tensor(
                                        pacc[:, b, :], pacc[:, b, :],
                                        ps_pool[:, :], OP.add)

                        # every NWIN/NQ windows: project the finished quarter
                        # of the NEXT layer and AllGather its xl chunk
                        if l < 2 and (w + 1) % (NWIN // NQ) == 0:
                            q = w // (NWIN // NQ)
                            hTq = hnpool.tile([128, KBn, QROWS], dt.bfloat16,
                                              tag="hTq")
                            for b in range(KBn):
                                nc.sync.dma_start(
                                    out=hTq[:, b, :],
                                    in_=h_dram[l][q * QROWS:(q + 1) * QROWS,
                                                  b * 128:(b + 1) * 128],
                                    transpose=True)
                            proj_quarter(l + 1, q, hTq, 0, wln, wrn,
                                         psAn, mmnpool)

                    if l == 2:
                        for b in range(2):
                            nc.sync.dma_start(
                                out=pool_loc[b * 128:(b + 1) * 128, :],
                                in_=pacc[:, b, :])

            if DEBUG:
                nc.sync.dma_start(out=dbg_xg0[0:GR_BASE, :],
                                  in_=xl_full[0][:, :])
                nc.sync.dma_start(out=dbg_xg0[GR_BASE:GR_BASE + NPAD, :],
                                  in_=xr_loc[0][:, :])
                nc.sync.dma_start(out=dbg_h0[:, :], in_=h_dram[0][:, :])
                nc.sync.dma_start(out=dbg_h1[:, :], in_=h_dram[1][:, :])

            # ---- pooled -> AllReduce -> MLP ----
            with (
                tc.tile_pool(name="mlp", bufs=1) as mpool,
                tc.tile_pool(name="psM", bufs=1, space="PSUM") as psM,
            ):
                nc.gpsimd.collective_compute(
                    "AllReduce", mybir.AluOpType.add, replica_groups=rg,
                    ins=[pool_loc.ap().opt()],
                    outs=[pool_full.ap().opt()])
                if DEBUG:
                    nc.sync.dma_start(out=dbg_pool[:, :], in_=pool_full[:, :])

                rcnt_t = mpool.tile([128, N_GRAPHS], dt.float32)
                nc.sync.dma_start(out=rcnt_t[:, :], in_=rcnt_in[:, :])
                pz = mpool.tile([128, 2, N_GRAPHS], dt.bfloat16)
                for b in range(2):
                    pf = mpool.tile([128, N_GRAPHS], dt.float32, tag="pf")
                    nc.sync.dma_start(out=pf[:, :],
                                      in_=pool_full[b * 128:(b + 1) * 128, :])
                    nc.vector.tensor_tensor(pz[:, b, :], pf[:, :],
                                            rcnt_t[:, :], OP.mult)

                wfc1_t = mpool.tile([128, 2, 256], dt.bfloat16)
                wfc2_t = mpool.tile([128, 2, 768], dt.bfloat16)
                for b in range(2):
                    nc.sync.dma_start(out=wfc1_t[:, b, :],
                                      in_=wfc1_in[:, b * 256:(b + 1) * 256])
                    nc.sync.dma_start(out=wfc2_t[:, b, :],
                                      in_=wfc2_in[:, b * 768:(b + 1) * 768])
                bfc1_t = mpool.tile([128, 2], dt.float32)
                nc.sync.dma_start(out=bfc1_t[:, :], in_=bfc1_in[:, :])
                bfc2_t = mpool.tile([128, 768], dt.float32)
                nc.sync.dma_start(out=bfc2_t[:, :], in_=bfc2_in[:, :])

                z1 = mpool.tile([128, 2, N_GRAPHS], dt.bfloat16)
                for it in range(2):
                    ps1 = psM.tile([128, N_GRAPHS], dt.float32, tag="ps1")
                    for b in range(2):
                        nc.tensor.matmul(
                            ps1[:, :],
                            wfc1_t[:, b, it * 128:(it + 1) * 128],
                            pz[:, b, :], start=(b == 0), stop=(b == 1))
                    nc.scalar.activation(z1[:, it, :], ps1[:, :], AF.Relu,
                                         bias=bfc1_t[:, it:it + 1], scale=1.0)

                for gt in range(N_GRAPHS // 128):
                    ps2 = psM.tile([128, 768], dt.float32, tag="ps2")
                    for jc, (j0, jw) in enumerate(((0, 512), (512, 256))):
                        for b in range(2):
                            nc.tensor.matmul(
                                ps2[:, j0:j0 + jw],
                                z1[:, b, gt * 128:(gt + 1) * 128],
                                wfc2_t[:, b, j0:j0 + jw],
                                start=(b == 0), stop=(b == 1))
                    zo = mpool.tile([128, 768], dt.float32, tag="zo")
                    nc.vector.tensor_tensor(zo[:, :], ps2[:, :],
                                            bfc2_t[:, :], OP.add)
                    nc.sync.dma_start(
                        out=out_ext[gt * 128:(gt + 1) * 128, :], in_=zo[:, :])

    nc.compile()
    return nc


def kernel(**inputs):
    T, in_maps = _preprocess(inputs)
    if T not in _PROG_CACHE:
        _PROG_CACHE[T] = _build(T)
    nc = _PROG_CACHE[T]
    r = run_bass_kernel_spmd(nc, in_maps, list(range(NCORES)), trace=False)
    return r.results[0]["out"]
